# revision 31
# baseline (speedup 1.0000x reference)
"""Trainium2 Bass kernel for a 6-layer dense transformer encoder.

Model: seq=4096, d_model=512, 8 heads x d_k=128, d_ff=1024, 6 layers,
post-LN residual blocks (LN after attention+residual and after FF+residual).

Sharding (8 NeuronCores, sequence-parallel): each core owns 512 sequence rows.
Per layer each core computes Q/K/V for its own rows for ALL heads; K and V are
all-gathered across cores in head-group chunks (the only collectives,
overlapped with QKV compute and attention); attention, Wo, the FFN and both
LayerNorms are computed entirely locally on the core's 512 rows with full
(replicated) weights. The final output is each core's 512 rows, concatenated
host-side.

v3 performance structure (on top of the v2 fp8/DoubleRow design):
- softmax exp is split across TWO engines: ACT computes exact exp for most
  score tiles; the DVE computes the rest via a Schraudolph-style trick
  (bits = round(8*(log2e*x + 7 - c)) as int8, bitcast to fp8e4), removing the
  ACT-pacing stalls that previously gated the attention inner loop.
- K readback stays fp8 (scores run mixed fp8 lhsT x bf16 qT rhs, which the PE
  supports exactly), halving K readback bytes.
- All post-AllGather readbacks are issued on the SP (sync) HWDGE queue instead
  of the gpsimd SWDGE queue, so they are not serialized behind the next
  chunk's descriptor setup (this removed a ~12us bubble per layer).
- Rowsum one-hot lhsT tiles are [*, 2, 16] instead of [*, 2, 128]: 8x less
  LDWEIGHTS traffic on the rowsum matmul stream.
- The x -> x^T transposes between blocks run on the DMA transpose crossbar
  (free wrt PE) from a bf16 staging copy instead of fp32 PE transposes.
- Residual stream is kept pre-scaled (input x2048 on host) so fp8/bf16 weight
  scaling folds into LayerNorm for free: LN(c*y) = LN(y), and the output
  scale B is folded into rstd via exp(-0.5*ln(var+eps') + ln(A*B)).
- Weight pools are double-buffered; next layer's QKV/Wo weights prefetch
  during the current layer's attention.

Attention runs in the scores-transposed layout: sT[k_idx, q_idx] so softmax
rowsums reduce over the PSUM partition axis via one-hot matmuls and
ctx^T = V-tile^T @ exp(sT) comes out ready for the Wo matmul with no
transposes of the 4096x4096 score matrix.
"""

import sys as _sys
import types as _types

import numpy as np

# Defensive: concourse's trace path imports antenv.axon_hooks, which this image
# lacks. Provide a no-op shim so an externally-set BASS_TRACE can't crash us.
if "antenv.axon_hooks" not in _sys.modules:
    _hm = _types.ModuleType("antenv.axon_hooks")
    _hm._hook = None
    _hm.set_axon_ntff_profile_hook = lambda h: setattr(_hm, "_hook", h)
    _hm.get_axon_ntff_profile_hook = lambda: _hm._hook
    _sys.modules["antenv.axon_hooks"] = _hm
    try:
        from trn_agent_boot.trn_boot import _ntff_profile_via_ctypes
        _hm.set_axon_ntff_profile_hook(
            _ntff_profile_via_ctypes("/opt/axon/libaxon_pjrt.so"))
    except Exception:
        pass

import concourse.bass as bass
import concourse.tile as tile
from concourse import bacc, mybir
from concourse.bass import ds, ts
from concourse import bass_utils as _bass_utils
from concourse.bass_utils import run_bass_kernel_spmd

# Defensive: the trace path uploads artifacts to a fish bucket that doesn't
# exist in this container; make it a no-op.
_bass_utils.upload_artifacts = lambda d: d
from concourse.masks import make_identity

# ---- force all activations into the one table set that has exp+ln+copy,
# so the whole kernel needs a single ACT_TABLE_LOAD instead of thrashing.
import concourse.bacc as _bacc_mod

_orig_get_tables = _bacc_mod.get_activation_tables


def _patched_get_tables(arch):
    tabs = _orig_get_tables(arch)
    if "natural_log_exp_and_others" in tabs:
        keep = tabs["natural_log_exp_and_others"]
        tabs = {
            name: (fns if name == "natural_log_exp_and_others" else set())
            for name, fns in tabs.items()
        }
        tabs["natural_log_exp_and_others"] = keep
    return tabs


_bacc_mod.get_activation_tables = _patched_get_tables

# model dims (hardcoded per problem spec)
L = 6          # layers
S = 4096       # sequence
C = 512        # d_model
H = 8          # heads
DK = 128       # head dim
DF = 1024      # d_ff
R = 8          # cores / ranks
SL = S // R    # local rows per core = 512
P = 128        # partitions
NT = SL // P   # local row tiles = 4
CT = C // P    # d_model tiles = 4
FT = DF // P   # d_ff tiles = 8
GT = S // P    # global row tiles = 32
EPS = 1e-5
SCALE = 1.0 / np.sqrt(DK)

# scaling scheme: host multiplies x by XS; fp8 weights by WS; W2 (bf16) by WS.
WS = 32.0              # fp8 weight scale
XS = WS * WS * 2.0     # residual-stream scale = 2048 (= ctx8-scale*WS)
FS = WS * WS           # FFN-block residual scale = 1024
LN2 = float(np.log(2.0))
LOG2E = float(np.log2(np.e))

# Schraudolph int8 fast-exp constants: for DVE-computed exp tiles,
# fp8e4 bits = round(8*(log2e*scale*x + 7 - C_SCH)); bitcast int8 -> fp8e4.
C_SCH = 0.0430

F32 = mybir.dt.float32
BF16 = mybir.dt.bfloat16
FP8 = mybir.dt.float8e4
I8 = mybir.dt.int8
AF = mybir.ActivationFunctionType
ALU = mybir.AluOpType
DR = mybir.MatmulPerfMode.DoubleRow

# which score-tile indices (tp in 0..15) get DVE fast-exp instead of ACT exp
DVE_TPS = frozenset((2, 5, 8, 11, 13, 15))


def _layer_norm_residual(nc, misc, y_f32, x_out_ap, eps_t, lnab_t):
    """x_out = B * LN(y) where y (already incl. residual) is pre-scaled by A.

    eps_t must hold A^2*EPS and lnab_t holds ln(A*B); then
    (y - mu) * exp(-0.5*ln(var + A^2 eps) + ln(A*B)) == B * LN(y/A).
    """
    stats = misc.tile([P, 6], F32, tag="stats")
    nc.vector.bn_stats(out=stats[:], in_=y_f32[:])
    mv = misc.tile([P, 2], F32, tag="mv")
    nc.vector.bn_aggr(out=mv[:], in_=stats[:])
    lnv = misc.tile([P, 1], F32, tag="lnv")
    nc.scalar.activation(out=lnv[:], in_=mv[:, 1:2], func=AF.Ln, bias=eps_t[:])
    rstd = misc.tile([P, 1], F32, tag="rstd")
    nc.scalar.activation(out=rstd[:], in_=lnv[:], func=AF.Exp, scale=-0.5,
                         bias=lnab_t[:])
    nc.vector.tensor_scalar(
        out=x_out_ap, in0=y_f32[:], scalar1=mv[:, 0:1], scalar2=rstd[:],
        op0=ALU.subtract, op1=ALU.mult,
    )


def build(n_cores=R):
    nc = bacc.Bacc("TRN2", target_bir_lowering=False, debug=False,
                   num_devices=n_cores)

    x_ext = nc.dram_tensor("x", [SL, C], F32, kind="ExternalInput")
    xb_ext = nc.dram_tensor("xb", [SL, C], BF16, kind="ExternalInput")
    wq_ext = nc.dram_tensor("wq", [L, C, H * DK], FP8, kind="ExternalInput")
    wk_ext = nc.dram_tensor("wk", [L, C, H * DK], FP8, kind="ExternalInput")
    wv_ext = nc.dram_tensor("wv", [L, C, H * DK], FP8, kind="ExternalInput")
    wo_ext = nc.dram_tensor("wo", [L, H * DK, C], FP8, kind="ExternalInput")
    w1_ext = nc.dram_tensor("w1", [L, C, DF], BF16, kind="ExternalInput")
    w2_ext = nc.dram_tensor("w2", [L, DF, C], BF16, kind="ExternalInput")
    bc_ext = nc.dram_tensor("bc", [8, H * P], BF16, kind="ExternalInput")
    out_ext = nc.dram_tensor("out", [SL, C], F32, kind="ExternalOutput")

    rg = [list(range(n_cores))]

    import contextlib
    with tile.TileContext(nc) as tc, contextlib.ExitStack() as es:
            consts = es.enter_context(tc.tile_pool(name="consts", bufs=1))
            xstate = es.enter_context(tc.tile_pool(name="xstate", bufs=2))
            xbp = es.enter_context(tc.tile_pool(name="xbp", bufs=2))
            xtp = es.enter_context(tc.tile_pool(name="xtp", bufs=2))
            xt8p = es.enter_context(tc.tile_pool(name="xt8p", bufs=2))
            wqkv = es.enter_context(tc.tile_pool(name="wqkv", bufs=2))
            wop = es.enter_context(tc.tile_pool(name="wop", bufs=2))
            wff = es.enter_context(tc.tile_pool(name="wff", bufs=1))
            qkvloc = es.enter_context(tc.tile_pool(name="qkvloc", bufs=1))
            gath = es.enter_context(tc.tile_pool(name="gath", bufs=2))
            expp = es.enter_context(tc.tile_pool(name="expp", bufs=10))
            ctxp = es.enter_context(tc.tile_pool(name="ctxp", bufs=1))
            hpool = es.enter_context(tc.tile_pool(name="hpool", bufs=1))
            misc = es.enter_context(tc.tile_pool(name="misc", bufs=4))
            psc = es.enter_context(tc.tile_pool(name="psc", bufs=2, space="PSUM"))
            pctx = es.enter_context(tc.tile_pool(name="pctx", bufs=2, space="PSUM"))
            prsp = es.enter_context(tc.tile_pool(name="prs", bufs=1, space="PSUM"))
            dram = es.enter_context(tc.tile_pool(name="dram", bufs=2, space="DRAM"))
            # bf16 identity for PE transposes (bf16 moving operand = fast path)
            ident = consts.tile([P, P], BF16)
            make_identity(nc, ident[:])
            # broadcast selectors: onehB[:, h, :] is [8, P] with row h all-ones,
            # so onehB[:, h, :].T @ recip[8, N] replicates recip row h onto all
            # 128 partitions (weights load from partition 0, which the BIR
            # verifier requires). Loaded from host: partition-offset memsets
            # are rejected by the verifier.
            onehB = consts.tile([8, H, P], BF16)
            nc.sync.dma_start(out=onehB[:], in_=bc_ext.rearrange("e (h p) -> e h p", p=P))
            eps_a = consts.tile([P, 1], F32)
            nc.vector.memset(eps_a[:], EPS * XS * XS)
            eps_f = consts.tile([P, 1], F32)
            nc.vector.memset(eps_f[:], EPS * FS * FS)
            # LN output-scale biases: (y_s - mu_s)*exp(-0.5*ln(var_s + A^2 eps))
            # is already unit LN(y) (the input scale A cancels), so the bias
            # is ln(B) for output scale B.
            lnb_attn = consts.tile([P, 1], F32)
            nc.vector.memset(lnb_attn[:], float(np.log(FS)))
            lnb_ff = consts.tile([P, 1], F32)
            nc.vector.memset(lnb_ff[:], float(np.log(XS)))
            lnb_last = consts.tile([P, 1], F32)
            nc.vector.memset(lnb_last[:], 0.0)
            ln2_t = consts.tile([8, 1], F32)
            nc.vector.memset(ln2_t[:], LN2)
            # one-hot fp8 lhsT tiles: head h -> rowsum lands on out row h.
            # only 16 columns (out partitions 0..15) to keep LDWEIGHTS tiny;
            # DR needs the Ko stride to be a multiple of 16 bytes.
            oneh8 = consts.tile([P, H, 2, 16], FP8)
            nc.vector.memset(oneh8[:], 0.0)
            for h in range(H):
                nc.vector.memset(oneh8[:, h, :, h:h + 1], 1.0)

            # layer-0 weights
            wq_sb = {}
            wk_sb = {}
            wv_sb = {}
            wo_sb = {}

            def _load_qkvo(l, split=False):
                wk_sb[l] = wqkv.tile([P, CT, H * DK], FP8, tag="wk", name=f"wk{l}")
                wk_src = wk_ext[l].rearrange("(ct p) n -> p ct n", p=P)
                wv_sb[l] = wqkv.tile([P, CT, H * DK], FP8, tag="wv", name=f"wv{l}")
                wv_src = wv_ext[l].rearrange("(ct p) n -> p ct n", p=P)
                if split:
                    # deadline-ordered first-layer loads: wk head 0 gates the
                    # first AllGather wave, then wv half 0 (v heads 0-3), ...
                    nc.scalar.dma_start(out=wk_sb[l][:, :, 0:DK], in_=wk_src[:, :, 0:DK])
                    nc.scalar.dma_start(out=wv_sb[l][:, :, 0:512], in_=wv_src[:, :, 0:512])
                    nc.scalar.dma_start(out=wk_sb[l][:, :, DK:], in_=wk_src[:, :, DK:])
                    nc.scalar.dma_start(out=wv_sb[l][:, :, 512:], in_=wv_src[:, :, 512:])
                else:
                    nc.scalar.dma_start(out=wk_sb[l][:], in_=wk_src)
                    nc.scalar.dma_start(out=wv_sb[l][:], in_=wv_src)
                wq_sb[l] = wqkv.tile([P, CT, H * DK], FP8, tag="wq", name=f"wq{l}")
                nc.scalar.dma_start(out=wq_sb[l][:], in_=wq_ext[l].rearrange("(ct p) n -> p ct n", p=P))
                wo_sb[l] = wop.tile([P, H, C], FP8, tag="wo", name=f"wo{l}")
                nc.scalar.dma_start(out=wo_sb[l][:], in_=wo_ext[l].rearrange("(h p) c -> p h c", p=P))

            _load_qkvo(0, split=True)

            def _tpose(xb_t, dst, st):
                """PE transpose (bf16 fast path) of one st row-block of the
                bf16 staging copy into dst[:, j, st-block] (any dtype)."""
                pt = prsp.tile([P, 4, P], BF16, tag="pt", name=f"pt{st}")
                for j in range(CT):
                    nc.tensor.transpose(pt[:, j, :], xb_t[:, st, ts(j, P)],
                                        ident[:])
                for j in range(CT):
                    nc.vector.tensor_copy(out=dst[:, j, ts(st, P)],
                                          in_=pt[:, j, :])

            # layer-0 transposed input: host provides the bf16 copy directly
            xb0 = xbp.tile([P, NT, C], BF16, tag="xb", name="xb_0")
            nc.sync.dma_start(out=xb0[:], in_=xb_ext.rearrange("(st p) c -> p st c", p=P))
            xT8_t = {}
            xT8_t[0] = xt8p.tile([P, CT, SL], FP8, tag="xT8", name="xT8_0")
            for st in range(NT):
                _tpose(xb0, xT8_t[0], st)
            # initial x state: [P, NT, C] f32 (pre-scaled x XS on host); only
            # needed at the Wo residual, so it loads after xb0
            x_cur = xstate.tile([P, NT, C], F32, tag="x")
            nc.sync.dma_start(out=x_cur[:], in_=x_ext.rearrange("(st p) c -> p st c", p=P))

            for l in range(L):
                wq_l, wk_l, wv_l, wo_l = wq_sb[l], wk_sb[l], wv_sb[l], wo_sb[l]
                xT8 = xT8_t[l]

                # ---- K^T then V (fp8 DoubleRow); K and V slices for one wave
                # are packed into ONE flat staging tile so a single AllGather
                # carries both (one collective latency per wave). ----
                kT = qkvloc.tile([P, H, SL], FP8, tag="kT", name=f"kT{l}")
                v_loc = qkvloc.tile([P, H, NT, DK], FP8, tag="vloc", name=f"vl{l}")
                HE = DK * SL  # elements per head slice
                # waves: (k-heads, v-heads). v0 must arrive with k0: the ctx
                # drain consumes vf h0 ~5us into attention.
                WAVES = [([0], [0]), ([1], [1]), ([2, 3], [2, 3]),
                         ([4, 5], [4, 5]), ([6, 7], [6, 7])]
                kv_in = {}
                k_outs = {}
                v_outs = {}

                def _kT_head(h, kT=kT, wk_l=wk_l, xT8=xT8):
                    pk = psc.tile([P, 2, SL], F32, tag="s", name=f"pk{h}")
                    for cp in range(2):
                        nc.tensor.matmul(pk[:, 0, :], wk_l[:, 2 * cp:2 * cp + 2, ts(h, DK)],
                                         xT8[:, 2 * cp:2 * cp + 2, :],
                                         start=(cp == 0), stop=(cp == 1), perf_mode=DR)
                    nc.scalar.activation(out=kT[:, h, :], in_=pk[:, 0, :], func=AF.Copy)

                def _v_mms(half, v_loc=v_loc, wv_l=wv_l, xT8=xT8):
                    for sp in range(2):
                        pv = psc.tile([P, 2, SL], F32, tag="s", name=f"pv{sp}")
                        for u in range(2):
                            si = 2 * sp + u
                            for cp in range(2):
                                nc.tensor.matmul(pv[:, u, :], xT8[:, 2 * cp:2 * cp + 2, ts(si, P)],
                                                 wv_l[:, 2 * cp:2 * cp + 2, ds(half * 512, 512)],
                                                 start=(cp == 0), stop=(cp == 1), perf_mode=DR)
                            nc.scalar.activation(
                                out=v_loc[:, ds(half * 4, 4), si, :],
                                in_=pv[:, u, :].rearrange("p (h d) -> p h d", d=DK),
                                func=AF.Copy)

                def _ag_wave(w, l=l, kT=kT, v_loc=v_loc):
                    kh, vh = WAVES[w]
                    n = len(kh) + len(vh)
                    kvi = dram.tile([n, HE], FP8, tag=f"kv_in{w}", name=f"kvi{l}_{w}")
                    nc.gpsimd.dma_start(
                        out=kvi[0:len(kh)].rearrange("h (d s) -> d h s", s=SL),
                        in_=kT[:, kh[0]:kh[0] + len(kh), :])
                    if vh:
                        nc.gpsimd.dma_start(
                            out=kvi[len(kh):n].rearrange("h (si sp d) -> sp h si d",
                                                         sp=P, d=DK),
                            in_=v_loc[:, vh[0]:vh[0] + len(vh)])
                    kvo = dram.tile([R, n, HE], FP8, tag=f"kv_out{w}",
                                    name=f"kvo{l}_{w}", addr_space="Shared")
                    nc.gpsimd.collective_compute(
                        "AllGather", ALU.bypass, replica_groups=rg,
                        ins=[kvi[:]], outs=[kvo[:]])
                    for i, h in enumerate(kh):
                        k_outs[h] = (kvo, i)
                    for i, h in enumerate(vh):
                        v_outs[h] = (kvo, len(kh) + i)

                # issue order tuned to per-head consumption deadlines
                _kT_head(0); _v_mms(0); _ag_wave(0)
                _kT_head(1); _ag_wave(1)
                _kT_head(2); _kT_head(3); _ag_wave(2)
                _kT_head(4); _kT_head(5)
                _v_mms(1); _ag_wave(3)
                _kT_head(6); _kT_head(7); _ag_wave(4)

                # FFN weights (bufs=1: emitted after the AG staging writes so
                # their HBM traffic never delays the first collective)
                w1_sb = wff.tile([P, CT, DF], BF16, tag="w1", name=f"w1_{l}")
                nc.scalar.dma_start(out=w1_sb[:], in_=w1_ext[l].rearrange("(ct p) n -> p ct n", p=P))
                w2_sb = wff.tile([P, FT, C], BF16, tag="w2", name=f"w2_{l}")
                nc.scalar.dma_start(out=w2_sb[:], in_=w2_ext[l].rearrange("(ft p) c -> p ft c", p=P))

                # ---- Q^T bf16 for all heads (overlaps the AllGathers) ----
                qT = qkvloc.tile([P, H, SL], BF16, tag="qT", name=f"qT{l}")
                for hp in range(4):
                    pq = psc.tile([P, 2, SL], F32, tag="s", name=f"pq{hp}")
                    for u in range(2):
                        h = 2 * hp + u
                        for cp in range(2):
                            nc.tensor.matmul(pq[:, u, :], wq_l[:, 2 * cp:2 * cp + 2, ts(h, DK)],
                                             xT8[:, 2 * cp:2 * cp + 2, :],
                                             start=(cp == 0), stop=(cp == 1), perf_mode=DR)
                        nc.vector.tensor_copy(out=qT[:, h, :], in_=pq[:, u, :])

                # prefetch next layer's QKV/Wo weights (bufs=2 pools)
                if l + 1 < L:
                    _load_qkvo(l + 1)

                # ---- attention, software-pipelined one step deep.
                # readbacks ride the SP HWDGE queue so they never queue behind
                # gpsimd descriptor setup for later AG chunks. ----
                def _readback(h, l=l):
                    kTf = gath.tile([P, R, SL], FP8, tag="kTf", name=f"kTf{l}_{h}")
                    ko, ki = k_outs[h]
                    nc.sync.dma_start(
                        out=kTf[:, 0:R // 2],
                        in_=ko[0:R // 2, ki].rearrange("r (d s) -> d r s", s=SL))
                    nc.sync.dma_start(
                        out=kTf[:, R // 2:R],
                        in_=ko[R // 2:R, ki].rearrange("r (d s) -> d r s", s=SL))
                    vf = gath.tile([P, R, NT, DK], FP8, tag="vf", name=f"vf{l}_{h}")
                    vo, vi = v_outs[h]
                    for r in range(R):
                        nc.sync.dma_start(
                            out=vf[:, r],
                            in_=vo[r, vi].rearrange("(si sp d) -> sp si d", sp=P, d=DK))
                    return kTf, vf

                ctxT_raw = ctxp.tile([P, H, SL], BF16, tag="ctxT", name=f"cr{l}")
                prs_t = prsp.tile([P, SL], F32, tag="rs", name=f"rs{l}")
                pend = []
                rs_pend = []

                def _drain(prs_t=prs_t, ctxT_raw=ctxT_raw):
                    h, tp, e_t, pctx_t, vf = pend.pop(0)
                    t0 = 2 * tp
                    e8 = e_t[:].bitcast(FP8)
                    nc.tensor.matmul(pctx_t[:], vf[:, t0 // NT, (t0 % NT):(t0 % NT) + 2, :],
                                     e8, start=(tp == 0), stop=(tp == GT // 2 - 1),
                                     perf_mode=DR)
                    rs_pend.append((h, tp, e_t))
                    if tp == GT // 2 - 1:
                        nc.vector.tensor_copy(out=ctxT_raw[:, h, :], in_=pctx_t[:])

                def _drain_rs(nmax, prs_t=prs_t):
                    for _ in range(min(nmax, len(rs_pend))):
                        h, tp, e_t = rs_pend.pop(0)
                        e8 = e_t[:].bitcast(FP8)
                        nc.tensor.matmul(prs_t[0:16, :], oneh8[:, h], e8,
                                         start=(h == 0 and tp == 0),
                                         stop=(h == H - 1 and tp == GT // 2 - 1),
                                         perf_mode=DR)

                A_sch = 8.0 * LOG2E * float(SCALE / (WS * WS))
                B_sch = 8.0 * (7.0 - C_SCH)

                rb = _readback(0)
                for h in range(H):
                    kTf, vf = rb
                    pctx_t = pctx.tile([P, SL], F32, tag="ctx", name=f"px{l}_{h}")
                    for tp in range(GT // 2):
                        pscore = psc.tile([P, 2, SL], F32, tag="s", name=f"ps{h}_{tp}")
                        for u in range(2):
                            t = 2 * tp + u
                            nc.tensor.matmul(pscore[:, u, :],
                                             kTf[:, t // NT, ts(t % NT, P)],
                                             qT[:, h, :], start=True, stop=True)
                        # keep the drain 2 deep: exp latency (~1.1us) exceeds
                        # one scores pair (~0.55us), so ctx one step behind
                        # would stall the PE on the exp semaphore every tile.
                        while len(pend) > 1:
                            _drain()
                        # rowsums paced 1 per score tile (matches the ctx
                        # production rate, so none pile up at the layer tail)
                        _drain_rs(1)
                        e_t = expp.tile([P, 2, SL], I8, tag="e", name=f"e{h}_{tp}")
                        if tp in DVE_TPS:
                            nc.vector.tensor_scalar(
                                out=e_t[:], in0=pscore[:],
                                scalar1=A_sch, scalar2=B_sch,
                                op0=ALU.mult, op1=ALU.add)
                        else:
                            nc.scalar.activation(out=e_t[:].bitcast(FP8), in_=pscore[:],
                                                 func=AF.Exp,
                                                 scale=float(SCALE / (WS * WS)))
                        pend.append((h, tp, e_t, pctx_t, vf))
                        if tp == 2 and h + 1 < H:
                            rb = _readback(h + 1)
                while pend:
                    _drain()
                _drain_rs(len(rs_pend))

                # ---- batched softmax reciprocals: 2 ACT ops for all heads ----
                lnrs = misc.tile([8, SL], F32, tag="lnrs", name=f"lr{l}")
                nc.scalar.activation(out=lnrs[:], in_=prs_t[0:8, :], func=AF.Ln)
                recip_sb = misc.tile([8, SL], BF16, tag="recip", name=f"rc{l}")
                nc.scalar.activation(out=recip_sb[:], in_=lnrs[:], func=AF.Exp,
                                     scale=-1.0, bias=ln2_t[:])
                ctxT8 = ctxp.tile([P, H, SL], FP8, tag="ctxT8", name=f"c8{l}")
                for hp in range(4):
                    pb = psc.tile([P, 2, SL], F32, tag="s", name=f"pb{hp}")
                    for u in range(2):
                        h = 2 * hp + u
                        nc.tensor.matmul(pb[:, u, :], onehB[:, h, :], recip_sb[:],
                                         start=True, stop=True)
                        nc.vector.tensor_mul(ctxT8[:, h, :], ctxT_raw[:, h, :], pb[:, u, :])

                # ---- Wo (fp8 DR) + residual + LN -> x2 (scale FS); bf16
                # staging copies feed the DMA-transpose crossbar ----
                x2 = xstate.tile([P, NT, C], F32, tag="x", name=f"x2_{l}")
                xb2 = xbp.tile([P, NT, C], BF16, tag="xb", name=f"xb2_{l}")
                x2T = xtp.tile([P, CT, SL], BF16, tag="xT", name=f"x2T{l}")
                for st in range(NT):
                    po = psc.tile([P, 2, SL], F32, tag="s", name=f"po{st}")
                    for i in range(4):
                        nc.tensor.matmul(po[:, 0, :], ctxT8[:, 2 * i:2 * i + 2, ts(st, P)],
                                         wo_l[:, 2 * i:2 * i + 2, :],
                                         start=(i == 0), stop=(i == 3), perf_mode=DR)
                    y = misc.tile([P, C], F32, tag="y", name=f"yo{st}")
                    nc.vector.tensor_add(y[:], po[:, 0, :], x_cur[:, st, :])
                    _layer_norm_residual(nc, misc, y, x2[:, st, :], eps_a, lnb_attn)
                    nc.vector.tensor_scalar_mul(xb2[:, st, :], x2[:, st, :], 1.0 / FS)
                    if st > 0:
                        _tpose(xb2, x2T, st - 1)
                _tpose(xb2, x2T, NT - 1)

                # ---- FF1 (bf16): hT = relu(W1^T x2^T) * WS ----
                hT = hpool.tile([P, FT, SL], BF16, tag="hT", name=f"hT{l}")
                for fp in range(4):
                    ph = psc.tile([P, 2, SL], F32, tag="s", name=f"ph{fp}")
                    for u in range(2):
                        f = 2 * fp + u
                        for c in range(CT):
                            nc.tensor.matmul(ph[:, u, :], w1_sb[:, c, ts(f, P)], x2T[:, c, :],
                                             start=(c == 0), stop=(c == CT - 1))
                        nc.vector.tensor_scalar(out=hT[:, f, :], in0=ph[:, u, :],
                                                scalar1=0.0, scalar2=WS,
                                                op0=ALU.max, op1=ALU.mult)

                # ---- FF2 (bf16, W2 x WS) + residual + LN -> x3; next-layer
                # transposes pipelined by st ----
                x3 = xstate.tile([P, NT, C], F32, tag="x", name=f"x3_{l}")
                last = (l == L - 1)
                if not last:
                    xb3 = xbp.tile([P, NT, C], BF16, tag="xb", name=f"xb3_{l}")
                    xT8_t[l + 1] = xt8p.tile([P, CT, SL], FP8, tag="xT8", name=f"xT8_{l + 1}")
                for st in range(NT):
                    pf = psc.tile([P, 2, SL], F32, tag="s", name=f"pf{st}")
                    for f in range(FT):
                        nc.tensor.matmul(pf[:, 0, :], hT[:, f, ts(st, P)], w2_sb[:, f, :],
                                         start=(f == 0), stop=(f == FT - 1))
                    y = misc.tile([P, C], F32, tag="y", name=f"yf{st}")
                    nc.vector.tensor_add(y[:], pf[:, 0, :], x2[:, st, :])
                    _layer_norm_residual(nc, misc, y, x3[:, st, :], eps_f,
                                         lnb_last if last else lnb_ff)
                    if not last:
                        nc.scalar.activation(out=xb3[:, st, :], in_=x3[:, st, :],
                                             func=AF.Copy, scale=1.0 / XS)
                        if st > 0:
                            _tpose(xb3, xT8_t[l + 1], st - 1)
                if not last:
                    _tpose(xb3, xT8_t[l + 1], NT - 1)

                x_cur = x3
            nc.sync.dma_start(out=out_ext.rearrange("(st p) c -> p st c", p=P), in_=x_cur[:])

    nc.compile()
    return nc


_NC_CACHE = {}


def _get_nc():
    if "nc" not in _NC_CACHE:
        _NC_CACHE["nc"] = build()
    return _NC_CACHE["nc"]


def kernel(enc_inputs, Wq, Wk, Wv, Wo, W1, W2, _trace=False):
    import ml_dtypes

    fp8_np = mybir.dt.np(FP8)
    x_raw = np.asarray(enc_inputs, dtype=np.float32).reshape(S, C)
    x = x_raw * XS
    xb = x_raw.astype(ml_dtypes.bfloat16)
    f8 = lambda a: (np.asarray(a, dtype=np.float32) * WS).astype(fp8_np)
    bf = lambda a, s=1.0: (np.asarray(a, dtype=np.float32) * s).astype(ml_dtypes.bfloat16)
    wq, wk, wv, wo = f8(Wq), f8(Wk), f8(Wv), f8(Wo)
    w1 = bf(W1)
    w2 = bf(W2, WS)
    bc = np.zeros((8, H, P), dtype=np.float32)
    for h in range(H):
        bc[h, h, :] = 1.0
    bc = bc.reshape(8, H * P).astype(ml_dtypes.bfloat16)

    in_maps = []
    for r in range(R):
        in_maps.append({
            "x": np.ascontiguousarray(x[r * SL:(r + 1) * SL]),
            "xb": np.ascontiguousarray(xb[r * SL:(r + 1) * SL]),
            "wq": wq, "wk": wk, "wv": wv, "wo": wo, "w1": w1, "w2": w2,
            "bc": bc,
        })

    nc = _get_nc()
    res = None
    last_err = None
    for _attempt in range(3):
        try:
            res = run_bass_kernel_spmd(nc, in_maps, core_ids=list(range(R)),
                                       trace=_trace)
            break
        except Exception as e:  # rare transient device-unrecoverable errors
            last_err = e
    if res is None:
        raise last_err
    out = np.concatenate([np.asarray(res.results[r]["out"]) for r in range(R)], axis=0)
    out = out.reshape(1, S, C).astype(np.float32)
    if _trace:
        return out, res
    return out
